# revision 5
# baseline (speedup 1.0000x reference)
"""Trainium2 Bass kernel for nn_MetaLearner (dual-branch GCN + PPMI meta-learner).

Strategy
--------
Host side: the edge-list GCN propagation is converted into a dense normalized
adjacency matmul:  gcn_prop(h) = A' @ h  with  A'[d,s] = norm[d]*norm[s]*#edges(s->d).
This makes both branches identical dense pipelines:

  H_L = relu(A'   @ relu(A'   @ (feats@W1L) + b1 ) @ W2L-prop...);  (see below)
  X   = feats @ [W1L | W1G]        (W1x = w1x @ tao_1_x, folded on host)
  H1b = relu(M_b @ X_b + b1_b)     M_L = A', M_G = PPMI
  Y_b = H1b @ W2b                  (W2b = w2b @ tao_2_b)
  H2b = relu(M_b @ Y_b + b2_b)
  a0  = sigmoid(concat(H2L,H2G) @ (W_a[:,0]-W_a[:,1]))    (softmax of 2 = sigmoid of diff)
  Z   = H2G + a0*(H2L - H2G);  out = Z @ W_c + b_c

Sharding: rows (nodes) are split 1024-per-core across 8 cores; A'/PPMI row-blocks
are passed per-core pre-transposed (K-major).  X is computed replicated (cheap),
Y is exchanged with one bf16 AllGather per branch.  All heavy matmuls run in
bf16 with fp32 PSUM accumulation, outputs transposed (features on partitions)
so biases are per-partition scalars.
"""

import sys

sys.path.insert(0, "/opt/trn_rl_repo")

import numpy as np
import ml_dtypes

import concourse.bacc as bacc
import concourse.mybir as mybir
import concourse.tile as tile
from concourse.bass_utils import run_bass_kernel_spmd

N = 8192
D_IN = 512
D_H = 256
D_O = 128
N_CLS = 8
CORES = 8
M_LOC = N // CORES          # 1024 rows per core
SK = N // 128               # 64 contraction chunks of 128
MB = M_LOC // 128           # 8 local row blocks
KC = D_IN // 128            # 4 k-chunks of input features
NB = D_H // 128             # 2 n-blocks of hidden features
F2 = 512                    # matmul free-dim slice
NH = M_LOC // F2            # 2 free-dim halves of the local rows

BF16 = mybir.dt.bfloat16
F32 = mybir.dt.float32
AF = mybir.ActivationFunctionType

_CACHE = {}


def _build():
    nc = bacc.Bacc("TRN2", target_bir_lowering=False, debug=False, num_devices=CORES)

    ftT_d = nc.dram_tensor("ftT", [D_IN, N], BF16, kind="ExternalInput")
    wb_d = nc.dram_tensor("w_both", [D_IN, 2 * D_H], BF16, kind="ExternalInput")
    w2l_d = nc.dram_tensor("w2l", [D_H, D_O], BF16, kind="ExternalInput")
    w2g_d = nc.dram_tensor("w2g", [D_H, D_O], BF16, kind="ExternalInput")
    a_d = nc.dram_tensor("a_t", [N, M_LOC], BF16, kind="ExternalInput")
    p_d = nc.dram_tensor("p_t", [N, M_LOC], BF16, kind="ExternalInput")
    b1_d = nc.dram_tensor("b1r", [128, NB], F32, kind="ExternalInput")
    b1g_d = nc.dram_tensor("b1gr", [128, NB], F32, kind="ExternalInput")
    b2_d = nc.dram_tensor("b2r", [D_O, 1], F32, kind="ExternalInput")
    b2g_d = nc.dram_tensor("b2gr", [D_O, 1], F32, kind="ExternalInput")
    bc_d = nc.dram_tensor("bcr", [N_CLS, 1], F32, kind="ExternalInput")
    wad_d = nc.dram_tensor("wad", [128, 2], BF16, kind="ExternalInput")  # [:,0]=L half, [:,1]=G half
    wc_d = nc.dram_tensor("w_c", [D_O, N_CLS], BF16, kind="ExternalInput")
    out_d = nc.dram_tensor("outT", [N_CLS, M_LOC], F32, kind="ExternalOutput")

    ftT_v = ftT_d[:].rearrange("(kc p) (sb sl) -> p kc sb sl", p=128, sl=128)
    a_v = a_d[:].rearrange("(c p) m -> p c m", p=128)
    p_v = p_d[:].rearrange("(c p) m -> p c m", p=128)

    with tile.TileContext(nc) as tc:
        with (
            tc.tile_pool(name="const", bufs=1) as cpool,
            tc.tile_pool(name="a_res", bufs=SK) as a_pool,
            tc.tile_pool(name="p_strm", bufs=8) as p_pool,
            tc.tile_pool(name="h1", bufs=1) as h1_pool,
            tc.tile_pool(name="ystage", bufs=2) as y_pool,
            tc.tile_pool(name="dram", bufs=1, space="DRAM") as dram,
        ):
            # ---- constants ----
            wb_s = cpool.tile([128, KC, 2 * D_H], BF16, name="wb_s")
            nc.sync.dma_start(wb_s[:], wb_d[:].rearrange("(kc p) m -> p kc m", p=128))
            w2l_s = cpool.tile([128, NB, D_O], BF16, name="w2l_s")
            nc.sync.dma_start(w2l_s[:], w2l_d[:].rearrange("(c p) m -> p c m", p=128))
            w2g_s = cpool.tile([128, NB, D_O], BF16, name="w2g_s")
            nc.sync.dma_start(w2g_s[:], w2g_d[:].rearrange("(c p) m -> p c m", p=128))
            b1_s = cpool.tile([128, NB], F32, name="b1_s")
            nc.sync.dma_start(b1_s[:], b1_d[:])
            b1g_s = cpool.tile([128, NB], F32, name="b1g_s")
            nc.sync.dma_start(b1g_s[:], b1g_d[:])
            b2_s = cpool.tile([D_O, 1], F32, name="b2_s")
            nc.sync.dma_start(b2_s[:], b2_d[:])
            b2g_s = cpool.tile([D_O, 1], F32, name="b2g_s")
            nc.sync.dma_start(b2g_s[:], b2g_d[:])
            bc_s = cpool.tile([N_CLS, 1], F32, name="bc_s")
            nc.sync.dma_start(bc_s[:], bc_d[:])
            wad_s = cpool.tile([128, 2], BF16, name="wad_s")
            nc.sync.dma_start(wad_s[:], wad_d[:])
            wc_s = cpool.tile([D_O, N_CLS], BF16, name="wc_s")
            nc.sync.dma_start(wc_s[:], wc_d[:])
            ones_s = cpool.tile([1, 128], BF16, name="ones_s")
            nc.gpsimd.memset(ones_s[:], 1.0)

            a_tiles = [a_pool.tile([128, M_LOC], BF16, name=f"a{s}", tag="a") for s in range(SK)]
            h1l = [h1_pool.tile([128, M_LOC], BF16, name=f"h1l{t}", tag=f"h1l{t}") for t in range(NB)]
            h1g = [h1_pool.tile([128, M_LOC], BF16, name=f"h1g{t}", tag=f"h1g{t}") for t in range(NB)]

            yl_bounce = dram.tile([M_LOC, D_O], BF16, name="yl_bounce")
            yg_bounce = dram.tile([M_LOC, D_O], BF16, name="yg_bounce")
            yl_all = dram.tile([N, D_O], BF16, addr_space="Shared", name="yl_all")
            yg_all = dram.tile([N, D_O], BF16, addr_space="Shared", name="yg_all")

            # ================= pass L: X = feats@[W1L|W1G]; prop1-L =================
            xg_ctx = tc.tile_pool(name="xg_res", bufs=SK)
            xg_pool = xg_ctx.__enter__()
            with (
                tc.tile_pool(name="ft", bufs=4) as ft_pool,
                tc.tile_pool(name="xlg", bufs=4) as x_pool,
                tc.tile_pool(name="ps_x", bufs=2, space="PSUM") as psx_pool,
                tc.tile_pool(name="ps_L", bufs=1, space="PSUM") as psl_pool,
            ):
                psum_L = [psl_pool.tile([128, M_LOC], F32, name=f"psl{t}", tag=f"psl{t}") for t in range(NB)]
                xg_tiles = [xg_pool.tile([128, D_H], BF16, name=f"xg{s}", tag="xg") for s in range(SK)]

                xl_prev = None
                for s in range(SK):
                    # stage 1: X chunk s
                    ft_s = ft_pool.tile([128, KC, 128], BF16, name=f"ft{s}", tag="ft")
                    nc.sync.dma_start(ft_s[:], ftT_v[:, :, s, :])
                    psx = psx_pool.tile([128, 2 * D_H], F32, name=f"psx{s}", tag="psx")
                    for k in range(KC):
                        nc.tensor.matmul(
                            psx[:], ft_s[:, k, :], wb_s[:, k, :],
                            start=(k == 0), stop=(k == KC - 1),
                        )
                    xl_s = x_pool.tile([128, D_H], BF16, name=f"xl{s}", tag="xl")
                    nc.vector.tensor_copy(xl_s[:], psx[:, 0:D_H])
                    nc.vector.tensor_copy(xg_tiles[s][:], psx[:, D_H:2 * D_H])
                    # A' chunk s load (stays resident for prop2-L)
                    nc.sync.dma_start(a_tiles[s][:], a_v[:, s, :])
                    # prop1-L for the previous chunk (software pipelining: PE has
                    # stage-1 work while DVE copies this chunk)
                    if xl_prev is not None:
                        sp, xp = xl_prev
                        for t in range(NB):
                            for h in range(NH):
                                nc.tensor.matmul(
                                    psum_L[t][:, h * F2:(h + 1) * F2],
                                    xp[:, t * 128:(t + 1) * 128],
                                    a_tiles[sp][:, h * F2:(h + 1) * F2],
                                    start=(sp == 0), stop=(sp == SK - 1),
                                )
                    xl_prev = (s, xl_s)
                sp, xp = xl_prev
                for t in range(NB):
                    for h in range(NH):
                        nc.tensor.matmul(
                            psum_L[t][:, h * F2:(h + 1) * F2],
                            xp[:, t * 128:(t + 1) * 128],
                            a_tiles[sp][:, h * F2:(h + 1) * F2],
                            start=(sp == 0), stop=(sp == SK - 1),
                        )
                # H1_L = relu(psum + b1) (transposed layout: features on partitions)
                for t in range(NB):
                    nc.scalar.activation(h1l[t][:], psum_L[t][:], AF.Relu, bias=b1_s[:, t:t + 1])

            # Y_L = H1_L @ W2L   (row-major out), bounce + AllGather
            with tc.tile_pool(name="ps_y", bufs=2, space="PSUM") as psy_pool:
                yst = y_pool.tile([128, MB, D_O], BF16, name="ylst", tag="yst")
                for mb in range(MB):
                    psy = psy_pool.tile([128, D_O], F32, name=f"psyl{mb}", tag="psy")
                    for t in range(NB):
                        nc.tensor.matmul(
                            psy[:], h1l[t][:, mb * 128:(mb + 1) * 128], w2l_s[:, t, :],
                            start=(t == 0), stop=(t == NB - 1),
                        )
                    nc.vector.tensor_copy(yst[:, mb, :], psy[:])
                for mb in range(MB):
                    nc.sync.dma_start(yl_bounce[mb * 128:(mb + 1) * 128, :], yst[:, mb, :])
            nc.gpsimd.collective_compute(
                "AllGather", mybir.AluOpType.bypass,
                ins=[yl_bounce.opt()], outs=[yl_all.opt()],
                replica_groups=[list(range(CORES))],
            )

            # ================= pass G: prop1-G over PPMI =================
            with tc.tile_pool(name="ps_G", bufs=1, space="PSUM") as psg_pool:
                psum_G = [psg_pool.tile([128, M_LOC], F32, name=f"psg{t}", tag=f"psg{t}") for t in range(NB)]
                p_tiles_1 = []
                for s in range(SK):
                    p_s = p_pool.tile([128, M_LOC], BF16, name=f"p1_{s}", tag="p")
                    nc.sync.dma_start(p_s[:], p_v[:, s, :])
                    p_tiles_1.append(p_s)
                    for t in range(NB):
                        for h in range(NH):
                            nc.tensor.matmul(
                                psum_G[t][:, h * F2:(h + 1) * F2],
                                xg_tiles[s][:, t * 128:(t + 1) * 128],
                                p_s[:, h * F2:(h + 1) * F2],
                                start=(s == 0), stop=(s == SK - 1),
                            )
                xg_ctx.__exit__(None, None, None)
                for t in range(NB):
                    nc.scalar.activation(h1g[t][:], psum_G[t][:], AF.Relu, bias=b1g_s[:, t:t + 1])

            with tc.tile_pool(name="ps_y2", bufs=2, space="PSUM") as psy_pool:
                yst = y_pool.tile([128, MB, D_O], BF16, name="ygst", tag="yst")
                for mb in range(MB):
                    psy = psy_pool.tile([128, D_O], F32, name=f"psyg{mb}", tag="psy")
                    for t in range(NB):
                        nc.tensor.matmul(
                            psy[:], h1g[t][:, mb * 128:(mb + 1) * 128], w2g_s[:, t, :],
                            start=(t == 0), stop=(t == NB - 1),
                        )
                    nc.vector.tensor_copy(yst[:, mb, :], psy[:])
                for mb in range(MB):
                    nc.sync.dma_start(yg_bounce[mb * 128:(mb + 1) * 128, :], yst[:, mb, :])
            nc.gpsimd.collective_compute(
                "AllGather", mybir.AluOpType.bypass,
                ins=[yg_bounce.opt()], outs=[yg_all.opt()],
                replica_groups=[list(range(CORES))],
            )

            # ================= prop2 (L from resident A', G restreams PPMI) =====
            with (
                tc.tile_pool(name="ygath", bufs=1) as g_pool,
                tc.tile_pool(name="epi", bufs=1) as e_pool,
            ):
                ylg = g_pool.tile([128, SK, D_O], BF16, name="ylg")
                nc.sync.dma_start(ylg[:], yl_all[:].rearrange("(c p) o -> p c o", p=128))
                ygg = g_pool.tile([128, SK, D_O], BF16, name="ygg")
                nc.sync.dma_start(ygg[:], yg_all[:].rearrange("(c p) o -> p c o", p=128))

                with tc.tile_pool(name="ps_2", bufs=1, space="PSUM") as ps2_pool:
                    ps_HL = ps2_pool.tile([128, M_LOC], F32, name="ps_HL")
                    for s in range(SK):
                        for h in range(NH):
                            nc.tensor.matmul(
                                ps_HL[:, h * F2:(h + 1) * F2],
                                ylg[:, s, :],
                                a_tiles[s][:, h * F2:(h + 1) * F2],
                                start=(s == 0), stop=(s == SK - 1),
                            )
                    hlt = e_pool.tile([128, M_LOC], BF16, name="hlt")
                    nc.scalar.activation(hlt[:], ps_HL[:], AF.Relu, bias=b2_s[:])

                    ps_HG = ps2_pool.tile([128, M_LOC], F32, name="ps_HG")
                    for s in range(SK):
                        p_s = p_pool.tile([128, M_LOC], BF16, name=f"p2_{s}", tag="p")
                        nc.sync.dma_start(p_s[:], p_v[:, s, :])
                        for h in range(NH):
                            nc.tensor.matmul(
                                ps_HG[:, h * F2:(h + 1) * F2],
                                ygg[:, s, :],
                                p_s[:, h * F2:(h + 1) * F2],
                                start=(s == 0), stop=(s == SK - 1),
                            )
                    hgt = e_pool.tile([128, M_LOC], BF16, name="hgt")
                    nc.scalar.activation(hgt[:], ps_HG[:], AF.Relu, bias=b2g_s[:])

                # ---- attention fusion + classifier (all in transposed layout) ----
                with tc.tile_pool(name="ps_3", bufs=1, space="PSUM") as ps3_pool:
                    ps_sd = ps3_pool.tile([1, M_LOC], F32, name="ps_sd")
                    for h in range(NH):
                        sl = slice(h * F2, (h + 1) * F2)
                        nc.tensor.matmul(ps_sd[:, sl], wad_s[:, 0:1], hlt[:, sl], start=True, stop=False)
                        nc.tensor.matmul(ps_sd[:, sl], wad_s[:, 1:2], hgt[:, sl], start=False, stop=True)
                    a0t = e_pool.tile([1, M_LOC], BF16, name="a0t")
                    nc.scalar.activation(a0t[:], ps_sd[:], AF.Sigmoid)

                    ps_a0 = ps3_pool.tile([128, M_LOC], F32, name="ps_a0")
                    for h in range(NH):
                        sl = slice(h * F2, (h + 1) * F2)
                        nc.tensor.matmul(ps_a0[:, sl], ones_s[:], a0t[:, sl], start=True, stop=True)

                    d_sb = e_pool.tile([128, M_LOC], BF16, name="d_sb")
                    nc.vector.tensor_sub(d_sb[:], hlt[:], hgt[:])
                    zt = e_pool.tile([128, M_LOC], BF16, name="zt")
                    nc.vector.tensor_mul(zt[:], d_sb[:], ps_a0[:])
                    nc.vector.tensor_add(zt[:], zt[:], hgt[:])

                    ps_out = ps3_pool.tile([N_CLS, M_LOC], F32, name="ps_out")
                    for h in range(NH):
                        sl = slice(h * F2, (h + 1) * F2)
                        nc.tensor.matmul(ps_out[:, sl], wc_s[:], zt[:, sl], start=True, stop=True)
                    out_sb = e_pool.tile([N_CLS, M_LOC], F32, name="out_sb")
                    nc.scalar.activation(out_sb[:], ps_out[:], AF.Identity, bias=bc_s[:])
                    nc.sync.dma_start(out_d[:], out_sb[:])

    nc.compile()
    return nc


def _prep(inputs):
    """Host-side preprocessing: fold tao into weights, build normalized dense
    adjacency from the edge list, pre-transpose / shard / cast operands."""
    f32 = np.float32
    bf = ml_dtypes.bfloat16
    feats = np.asarray(inputs["feats"], f32)
    norm = np.asarray(inputs["norm"], f32)
    PPMI = np.asarray(inputs["PPMI"], f32)
    src = np.asarray(inputs["src"]).astype(np.int64)
    dst = np.asarray(inputs["dst"]).astype(np.int64)

    w1L = np.asarray(inputs["w1"], f32) @ np.asarray(inputs["tao_1_L"], f32)
    w1G = np.asarray(inputs["w1g"], f32) @ np.asarray(inputs["tao_1_G"], f32)
    w2L = np.asarray(inputs["w2"], f32) @ np.asarray(inputs["tao_2_L"], f32)
    w2G = np.asarray(inputs["w2g"], f32) @ np.asarray(inputs["tao_2_G"], f32)
    W_a = np.asarray(inputs["W_a"], f32)
    W_c = np.asarray(inputs["W_c"], f32)

    # dense normalized adjacency, pre-transposed: AT[s, d] = norm[d]*norm[s]*count(s->d)
    nv = norm[:, 0]
    AT = np.zeros((N, N), f32)
    np.add.at(AT, (src, dst), nv[src] * nv[dst])
    AT_bf = AT.astype(bf)
    PT_bf = np.ascontiguousarray(PPMI.T).astype(bf)

    wad = (W_a[:, 0] - W_a[:, 1]).astype(f32)  # [256]

    common = {
        "ftT": np.ascontiguousarray(feats.T).astype(bf),
        "w_both": np.concatenate([w1L, w1G], axis=1).astype(bf),
        "w2l": w2L.astype(bf),
        "w2g": w2G.astype(bf),
        "b1r": np.asarray(inputs["b1"], f32).reshape(NB, 128).T.copy(),
        "b1gr": np.asarray(inputs["b1g"], f32).reshape(NB, 128).T.copy(),
        "b2r": np.asarray(inputs["b2"], f32).reshape(D_O, 1).copy(),
        "b2gr": np.asarray(inputs["b2g"], f32).reshape(D_O, 1).copy(),
        "bcr": np.asarray(inputs["b_c"], f32).reshape(N_CLS, 1).copy(),
        "wad": np.stack([wad[:128], wad[128:]], axis=1).astype(bf),
        "w_c": W_c.astype(bf),
    }
    in_maps = []
    for c in range(CORES):
        sel = slice(c * M_LOC, (c + 1) * M_LOC)
        m = dict(common)
        m["a_t"] = np.ascontiguousarray(AT_bf[:, sel])
        m["p_t"] = np.ascontiguousarray(PT_bf[:, sel])
        in_maps.append(m)
    return in_maps


def kernel(**inputs) -> np.ndarray:
    if "nc" not in _CACHE:
        _CACHE["nc"] = _build()
    nc = _CACHE["nc"]
    in_maps = _prep(inputs)
    res = run_bass_kernel_spmd(nc, in_maps, list(range(CORES)), trace=False)
    out = np.empty((N, N_CLS), np.float32)
    for c in range(CORES):
        out[c * M_LOC:(c + 1) * M_LOC, :] = res.results[c]["outT"].T
    return out


if __name__ == "__main__":
    rng = np.random.default_rng(0)
    dummy = {
        "feats": rng.standard_normal((N, D_IN)).astype(np.float32),
        "norm": rng.random((N, 1)).astype(np.float32),
        "tao_1_L": rng.standard_normal((D_H, D_H)).astype(np.float32) / 16,
        "tao_2_L": rng.standard_normal((D_O, D_O)).astype(np.float32) / 11,
        "tao_1_G": rng.standard_normal((D_H, D_H)).astype(np.float32) / 16,
        "tao_2_G": rng.standard_normal((D_O, D_O)).astype(np.float32) / 11,
        "PPMI": rng.random((N, N)).astype(np.float32) / N,
        "w1": rng.random((D_IN, D_H)).astype(np.float32) * 0.06,
        "b1": rng.random((D_H,)).astype(np.float32) * 0.04,
        "w2": rng.random((D_H, D_O)).astype(np.float32) * 0.09,
        "b2": rng.random((D_O,)).astype(np.float32) * 0.06,
        "w1g": rng.random((D_IN, D_H)).astype(np.float32) * 0.06,
        "b1g": rng.random((D_H,)).astype(np.float32) * 0.04,
        "w2g": rng.random((D_H, D_O)).astype(np.float32) * 0.09,
        "b2g": rng.random((D_O,)).astype(np.float32) * 0.06,
        "W_a": rng.random((2 * D_O, 2)).astype(np.float32) * 0.7,
        "W_c": rng.random((D_O, N_CLS)).astype(np.float32) * 0.35,
        "b_c": rng.random((N_CLS,)).astype(np.float32) * 0.35,
        "src": rng.integers(0, N, (262144,)).astype(np.int32),
        "dst": rng.integers(0, N, (262144,)).astype(np.int32),
    }
    out = kernel(**dummy)
    print("out", out.shape, out.dtype, np.abs(out).mean())
